# revision 8
# baseline (speedup 1.0000x reference)
"""AttentionAggregationV2 GNN message-passing kernel for 8 Trainium2 NeuronCores.

Strategy: shard by attention HEAD (8 heads -> 8 cores). Each head's edge
softmax and weighted aggregation are independent, so there are no
collectives. Per core the computation is reformulated as a single
segmented sum:

    out[n, c] = (sum_{e: dst[e]=n} exp(w_e) * v_e[c]) / (sum exp(w_e))

implemented as one-hot routing matmuls on the PE array: edges are grouped
(sorted by destination) into 128-node windows; for each chunk of 128 edges
the DVE builds a scaled one-hot matrix (iota == dst_lo) * exp(w) in a
single tensor_scalar instruction and the PE accumulates
onehot^T @ [v | 1] into the window's PSUM accumulator.

The max-subtraction in the reference edge-softmax is skipped: w =
cutoff * weight with cutoff in [0,1] and weight ~ N(0,1), so exp(w) is
comfortably inside f32 range and softmax(w) == softmax(w - max) exactly up
to rounding.
"""

import numpy as np
import ml_dtypes
from contextlib import ExitStack

import concourse.bass as bass
import concourse.bacc as bacc
import concourse.tile as tile
from concourse import mybir
from concourse.bass_utils import run_bass_kernel_spmd

N_NODES = 50000
N_EDGES = 800000
NUM_HEADS = 8
P = 128
NWIN = (N_NODES + P - 1) // P  # 391 windows of 128 nodes
PCOLS = 41                     # 40 value cols + 1 softmax-denominator col
GROUP = 64                     # pv chunks per streaming DMA group

# per-head output column ranges in the fused [N, 320] layout
HEAD_SIZES = [16, 24]
BLOCK_STARTS = [0, 128]

last_results = None  # BassKernelResults of the most recent run (for test.py)
last_nc = None
last_in_maps = None


def _head_cols(h):
    c0 = np.arange(BLOCK_STARTS[0] + 16 * h, BLOCK_STARTS[0] + 16 * (h + 1))
    c1 = np.arange(BLOCK_STARTS[1] + 24 * h, BLOCK_STARTS[1] + 24 * (h + 1))
    return np.concatenate([c0, c1])


def _build(chunks_per_window):
    """Build the SPMD Bass program. Identical for all cores; only data differs."""
    C = int(np.sum(chunks_per_window))
    dt = mybir.dt
    nc = bacc.Bacc(trn_type="TRN2")

    pv_d = nc.dram_tensor("pv", [P, C, PCOLS], dt.bfloat16, kind="ExternalInput")
    dstlo_d = nc.dram_tensor("dstlo", [P, C], dt.float32, kind="ExternalInput")
    cut_d = nc.dram_tensor("cut", [P, C], dt.float32, kind="ExternalInput")
    wgt_d = nc.dram_tensor("wgt", [P, C], dt.float32, kind="ExternalInput")
    out_d = nc.dram_tensor("out", [NWIN * P, PCOLS], dt.float32, kind="ExternalOutput")

    iota_np = np.tile(
        np.arange(P, dtype=np.float32).astype(ml_dtypes.bfloat16), (P, 1))
    iota_d = nc.inline_tensor(np.asarray(iota_np), name="iota")

    with tile.TileContext(nc) as tc:
        with ExitStack() as ctx:
            cpool = ctx.enter_context(tc.tile_pool(name="const", bufs=1))
            spool = ctx.enter_context(tc.tile_pool(name="stream", bufs=3))
            ohpool = ctx.enter_context(tc.tile_pool(name="oh", bufs=4))
            opool = ctx.enter_context(tc.tile_pool(name="outp", bufs=4))
            psum = ctx.enter_context(tc.tile_pool(name="ps", bufs=4, space="PSUM"))

            iota_t = cpool.tile([P, P], dt.bfloat16)
            nc.sync.dma_start(iota_t[:], iota_d[:])
            dstlo_t = cpool.tile([P, C], dt.float32)
            nc.sync.dma_start(dstlo_t[:], dstlo_d[:])
            cut_t = cpool.tile([P, C], dt.float32)
            nc.sync.dma_start(cut_t[:], cut_d[:])
            wgt_t = cpool.tile([P, C], dt.float32)
            nc.sync.dma_start(wgt_t[:], wgt_d[:])

            # w = cutoff * weight (DVE), e = exp(w) (ACT)
            t_t = cpool.tile([P, C], dt.float32)
            nc.vector.tensor_tensor(t_t[:], cut_t[:], wgt_t[:], mybir.AluOpType.mult)
            e_t = cpool.tile([P, C], dt.float32)
            nc.scalar.activation(e_t[:], t_t[:], mybir.ActivationFunctionType.Exp)

            zero_t = cpool.tile([P, PCOLS], dt.float32)
            nc.vector.memset(zero_t[:], 0.0)

            pv_t = None
            c = 0
            for w in range(NWIN):
                kw = int(chunks_per_window[w])
                if kw == 0:
                    nc.sync.dma_start(out_d[w * P:(w + 1) * P, :], zero_t[:])
                    continue
                acc = psum.tile([P, PCOLS], dt.float32)
                for j in range(kw):
                    g, off = divmod(c, GROUP)
                    if off == 0:
                        gsz = min(GROUP, C - g * GROUP)
                        pv_t = spool.tile([P, GROUP, PCOLS], dt.bfloat16, tag="pv")
                        nc.sync.dma_start(
                            pv_t[:, :gsz, :], pv_d[:, g * GROUP:g * GROUP + gsz, :])
                    oh = ohpool.tile([P, P], dt.bfloat16, tag="oh")
                    nc.vector.tensor_scalar(
                        oh[:], iota_t[:],
                        dstlo_t[:, c:c + 1], e_t[:, c:c + 1],
                        mybir.AluOpType.is_equal, mybir.AluOpType.mult)
                    nc.tensor.matmul(
                        acc[:], oh[:], pv_t[:, off, :],
                        start=(j == 0), stop=(j == kw - 1))
                    c += 1
                # flush raw (u, s) sums via ACT copy (PSUM can't DMA);
                # the u/s division happens on the host
                o_t = opool.tile([P, PCOLS], dt.float32, tag="o")
                nc.scalar.copy(o_t[:], acc[:])
                nc.sync.dma_start(out_d[w * P:(w + 1) * P, :], o_t[:])
            assert c == C
    nc.compile()
    return nc


def kernel(value, edge_weights, edge_weights_cutoff, edge_index,
           _trace=False, _trace_kwargs=None):
    global last_results, last_nc, last_in_maps
    value = np.asarray(value)
    edge_weights = np.asarray(edge_weights)
    cutoff = np.asarray(edge_weights_cutoff)
    dst = np.asarray(edge_index)[1].astype(np.int64)
    E = dst.shape[0]

    # ---- shard prep: sort edges by destination, pad each 128-node window
    # to a multiple of 128 edges so every chunk maps to one window ----
    order = np.argsort(dst, kind="stable")
    dsts = dst[order]
    win = (dsts >> 7).astype(np.int64)
    counts = np.bincount(win, minlength=NWIN)
    pc = ((counts + P - 1) // P) * P          # padded edges per window
    chunks_per_window = pc // P
    T = int(pc.sum())
    C = T // P

    pad_start = np.zeros(NWIN, np.int64)
    pad_start[1:] = np.cumsum(pc)[:-1]
    wstart = np.zeros(NWIN, np.int64)
    wstart[1:] = np.cumsum(counts)[:-1]
    pos = pad_start[win] + (np.arange(E) - wstart[win])

    src = np.zeros(T, np.int64)               # original edge id per slot
    valid = np.zeros(T, np.float32)
    dstlo = np.zeros(T, np.float32)
    src[pos] = order
    valid[pos] = 1.0
    dstlo[pos] = (dsts & 127).astype(np.float32)

    def to_pc(a):  # [T] -> [128, C] with slot t -> (t % 128, t // 128)
        return np.ascontiguousarray(a.reshape(C, P).T)

    dstlo_pc = to_pc(dstlo)
    cut_pc = to_pc(cutoff[src] * valid)       # pads: w=0 (e=1, killed by flag)
    v0 = value[:, :128].reshape(E, 8, 16)
    v1 = value[:, 128:].reshape(E, 8, 24)

    in_maps = []
    for h in range(NUM_HEADS):
        vh = np.concatenate([v0[:, h, :], v1[:, h, :]], axis=1)  # [E, 40]
        pv = np.zeros((T, PCOLS), np.float32)
        pv[:, :40] = vh[src] * valid[:, None]
        pv[:, 40] = valid
        pv_pc = np.ascontiguousarray(
            pv.reshape(C, P, PCOLS).transpose(1, 0, 2)).astype(ml_dtypes.bfloat16)
        wgt_pc = to_pc(edge_weights[src, h] * valid)
        in_maps.append({
            "pv": np.asarray(pv_pc),
            "dstlo": np.asarray(dstlo_pc),
            "cut": cut_pc.astype(np.float32),
            "wgt": wgt_pc.astype(np.float32),
        })

    nc = _build(chunks_per_window)
    last_nc, last_in_maps = nc, in_maps
    res = run_bass_kernel_spmd(
        nc, in_maps, core_ids=list(range(8)),
        trace=_trace, **(_trace_kwargs or {}))
    last_results = res

    out = np.zeros((N_NODES, 320), np.float32)
    for h in range(NUM_HEADS):
        us = res.results[h]["out"][:N_NODES]
        out[:, _head_cols(h)] = us[:, :40] / np.maximum(us[:, 40:41], 1e-30)
    return out
